# revision 2
# baseline (speedup 1.0000x reference)
"""Trainium2 Bass kernel for nn_DAttention: out[b,c,d,h,w] = x[b,c,d,h,w] * mean_{c,h,w}(x[b,:,d,:,:]).

Sharding: pure data parallel over batch B=8 -> one batch per NeuronCore.
Per core, loop over d (32 slices of 2 MiB): load x[b,:,d,:,:] into SBUF,
reduce to the scalar mean, multiply in SBUF, store. Single pass over HBM:
64 MiB read + 64 MiB written per core (the memory roofline).

SBUF layout per d-slice: tile [128, 4096] with partition p = c*4 + hg
(h split into 4 groups of 32), free = (h%32)*128 + w. Each partition row
is one contiguous 16 KiB run in DRAM -> near line-rate DMA descriptors.

Mean: DVE reduce_sum over the free axis -> [128,1] partial sums, then one
PE matmul against a constant 128x128 matrix filled with 1/524288 (exact
power of two) which both cross-partition-reduces and broadcasts the mean
to all 128 partitions in PSUM. ACT copies PSUM->SBUF; DVE tensor_scalar
multiplies the resident tile by the per-partition scalar.
"""
import numpy as np

import concourse.bacc as bacc
import concourse.tile as tile
import concourse.mybir as mybir
from concourse.bass_utils import run_bass_kernel_spmd

B, C, D, H, W = 8, 32, 32, 128, 128
HG, HL = 4, 32          # H split: partition dim = C*HG = 128
P = C * HG              # 128 partitions
F = HL * W              # 4096 free elements per partition
N_RED = C * H * W       # 524288 = 2**19 elements reduced per (b, d)
RECIP = 1.0 / N_RED     # exact in fp32

_NC = None


def _build_nc():
    nc = bacc.Bacc("TRN2", target_bir_lowering=False, debug=False)
    x5 = nc.dram_tensor("x", [C, D, HG, HL, W], mybir.dt.float32, kind="ExternalInput")
    o5 = nc.dram_tensor("out", [C, D, HG, HL, W], mybir.dt.float32, kind="ExternalOutput")
    with tile.TileContext(nc) as tc:
        with (
            tc.tile_pool(name="xin", bufs=3) as xpool,
            tc.tile_pool(name="oout", bufs=3) as opool,
            tc.tile_pool(name="small", bufs=4) as spool,
            tc.tile_pool(name="psum", bufs=2, space="PSUM") as ppool,
            tc.tile_pool(name="const", bufs=1) as cpool,
        ):
            recip = cpool.tile([P, P], mybir.dt.float32)
            nc.gpsimd.memset(recip[:], RECIP)
            for d in range(D):
                xt = xpool.tile([P, F], mybir.dt.float32)
                nc.sync.dma_start(xt[:], x5[:, d])
                cs = spool.tile([P, 1], mybir.dt.float32)
                nc.vector.reduce_sum(cs[:], xt[:], axis=mybir.AxisListType.X)
                dv = ppool.tile([P, 1], mybir.dt.float32)
                nc.tensor.matmul(dv[:], recip[:], cs[:])
                dvs = spool.tile([P, 1], mybir.dt.float32)
                nc.scalar.copy(dvs[:], dv[:])
                ot = opool.tile([P, F], mybir.dt.float32)
                nc.vector.tensor_scalar_mul(ot[:], xt[:], dvs[:])
                nc.scalar.dma_start(o5[:, d], ot[:])
    nc.compile()
    return nc


def _get_nc():
    global _NC
    if _NC is None:
        _NC = _build_nc()
    return _NC


def run(x: np.ndarray, trace: bool = False, tmpdir: str | None = None):
    """Run on 8 NeuronCores; returns (out, BassKernelResults)."""
    x = np.asarray(x)
    assert x.shape == (B, C, D, H, W), x.shape
    x = x.astype(np.float32, copy=False)
    nc = _get_nc()
    in_maps = [
        {"x": np.ascontiguousarray(x[b]).reshape(C, D, HG, HL, W)} for b in range(B)
    ]
    res = run_bass_kernel_spmd(
        nc, in_maps, core_ids=list(range(B)), trace=trace, tmpdir=tmpdir
    )
    out = np.stack([r["out"].reshape(C, D, H, W) for r in res.results])
    return out, res


def kernel(x: np.ndarray) -> np.ndarray:
    out, _ = run(x)
    return out
